# revision 9
# baseline (speedup 1.0000x reference)
"""GQA causal attention (B=2,S=2048,HID=2048,H=16,KVH=4,D=128) on 8 trn2 cores.

Sharding: core = b*4 + g  (b: batch, g: head-group of 4 Q heads + 1 KV head).
Per-core kernel computes q/k/v projections (+RoPE), causal softmax attention
for its 4 heads, and a partial output projection; host sums the 4 partials
per batch.

Layout strategy (all matmuls bf16 with fp32 PSUM accumulation):
  - hiddenT [HID, S] per batch; weights pre-transposed on host.
  - Projections (N=1024 streams) produce qT/kT/vT [dims, S]; RoPE applied in
    qT layout (rotate-half = partition-shifted ACT copy).
  - scoresT[sj, si] = kT_slice.T @ qT -> ACT exp (1/sqrt(D) folded into the
    activation scale; no max-subtraction: scores are O(+-10) so fp32 exp is
    safe) -> bf16 expT tiles.
  - AV with v stationary: avT[d, si] += v[sj,:].T @ expT[sj, si] — N<=1024
    streams, result lands directly in the xT layout needed by the output
    projection (no transposes).
  - softmax denominators: DVE-accumulate expT over sj tiles, then
    gpsimd.partition_all_reduce (replicated row sums) + reciprocal + mul.
  - out_p[s, :] = sum_m xT[m, s-block].T @ woT[m, :]  (N=1024).
"""

import math
import numpy as np
import ml_dtypes

B, S, HID = 2, 2048, 2048
H, KVH, D = 16, 4, 128
GROUPS = 4            # head groups == KV heads
HD_PER_G = 4          # query heads per group
N_CORES = 8
P = 128
HIDC = HID // P       # 16 hid chunks
W = 512               # matmul chunk (psum bank limit: 512 f32)
WC = S // W           # 4 s-chunks
SB = S // P           # 16 s-blocks of 128

BF16 = ml_dtypes.bfloat16
_CACHE = {}


def build_nc(repeat=1, loop_n=None, internal_inputs=False):
    """loop_n: if set, wrap the body in a hardware For_i loop (for timing).
    internal_inputs: declare inputs as internal DRAM (garbage data, no host
    upload) — timing-only variant."""
    import contextlib
    import concourse.bass as bass
    import concourse.tile as tile
    from concourse import bacc, mybir
    from concourse.bass_isa import ReduceOp

    f32 = mybir.dt.float32
    bf16 = mybir.dt.bfloat16

    nc = bacc.Bacc("TRN2", target_bir_lowering=False, debug=False,
                   num_devices=N_CORES)

    def din(name, shape, dt):
        if internal_inputs:
            return nc.dram_tensor(name, shape, dt).ap()
        return nc.dram_tensor(name, shape, dt, kind="ExternalInput").ap()
    hT = din("hT", [HID, S], bf16)
    wqT = din("wqT", [HID, HD_PER_G * D], bf16)
    wkT = din("wkT", [HID, D], bf16)
    wvT = din("wvT", [HID, D], bf16)
    woT = din("woT", [HD_PER_G * D, HID], bf16)
    cosT = din("cosT", [D, S], f32)
    sinT = din("sinT", [D, S], f32)
    maskT = din("maskT", [P, P], bf16)
    ident = din("ident", [P, P], bf16)
    if internal_inputs:
        out = nc.dram_tensor("out", [S, HID], f32).ap()
        sink = nc.dram_tensor("sink", [P, 4], f32, kind="ExternalOutput").ap()
    else:
        out = nc.dram_tensor("out", [S, HID], f32, kind="ExternalOutput").ap()
        sink = None

    inv_sqrt_d = 1.0 / math.sqrt(D)

    with tile.TileContext(nc) as tc:
        with (
            tc.tile_pool(name="consts", bufs=1) as consts,
            tc.tile_pool(name="persist", bufs=1) as persist,
            tc.tile_pool(name="hpool", bufs=2) as hpool,
            tc.tile_pool(name="rope", bufs=2) as rope,
            tc.tile_pool(name="expp", bufs=6) as expp,
            tc.tile_pool(name="dsump", bufs=2) as dsump,
            tc.tile_pool(name="rcp", bufs=1) as rcp,
            tc.tile_pool(name="outp", bufs=2) as outp,
            tc.tile_pool(name="ps512", bufs=5, space="PSUM") as ps512,
            tc.tile_pool(name="avt", bufs=2, space="PSUM") as avtp,
            tc.tile_pool(name="psx", bufs=1, space="PSUM") as psx,
        ):
            # ---- constant loads --------------------------------------
            wqT_sb = consts.tile([P, HIDC, HD_PER_G * D], bf16)
            nc.sync.dma_start(out=wqT_sb, in_=wqT.rearrange("(c p) d -> p c d", p=P))
            wkT_sb = consts.tile([P, HIDC, D], bf16)
            nc.sync.dma_start(out=wkT_sb, in_=wkT.rearrange("(c p) d -> p c d", p=P))
            wvT_sb = consts.tile([P, HIDC, D], bf16)
            nc.sync.dma_start(out=wvT_sb, in_=wvT.rearrange("(c p) d -> p c d", p=P))
            cosT_sb = consts.tile([P, S], f32)
            nc.sync.dma_start(out=cosT_sb, in_=cosT)
            sinT_sb = consts.tile([P, S], f32)
            nc.sync.dma_start(out=sinT_sb, in_=sinT)
            mask_sb = consts.tile([P, P], bf16)
            nc.sync.dma_start(out=mask_sb, in_=maskT)
            ident_sb = consts.tile([P, P], bf16)
            nc.sync.dma_start(out=ident_sb, in_=ident)
            woT_sb = consts.tile([P, HD_PER_G, HID], bf16)
            nc.sync.dma_start(out=woT_sb, in_=woT.rearrange("(m p) h -> p m h", p=P))

            # ---- persistent intermediates ----------------------------
            qrT_sb = persist.tile([P, HD_PER_G, S], bf16)   # rotated qT per head
            krT_sb = persist.tile([P, S], bf16)             # rotated kT
            vT_sb = persist.tile([P, S], bf16)              # vT (pre-transpose)
            v_nat = persist.tile([P, SB, D], bf16)          # v natural [sj, d]
            xT_sb = persist.tile([P, HD_PER_G, S], bf16)    # attn out (transposed)

            def rope_chunk(ps, dst_ap, c):
                """dst = ps*cos + rot_half(ps)*sin_signed on wide chunk c."""
                sl = slice(c * W, (c + 1) * W)
                t1 = rope.tile([P, W], f32, tag="t1")
                nc.vector.tensor_mul(t1, ps, cosT_sb[:, sl])
                t2 = rope.tile([P, W], f32, tag="t2")
                nc.vector.tensor_copy(t2[0:64, :], ps[64:128, :])
                nc.vector.tensor_copy(t2[64:128, :], ps[0:64, :])
                nc.vector.tensor_mul(t2, t2, sinT_sb[:, sl])
                nc.vector.tensor_add(dst_ap, t1, t2)

            if internal_inputs:
                # timing-only: fill internal inputs with finite values
                zb = consts.tile([P, S], bf16, tag="zb")
                nc.vector.memset(zb, 0.01)
                zf1 = consts.tile([P, S], f32, tag="zf1")
                nc.vector.memset(zf1, 1.0)
                zf0 = consts.tile([P, S], f32, tag="zf0")
                nc.vector.memset(zf0, 0.0)
                for cc in range(HIDC):
                    hrc = hT.rearrange("(c p) s -> c p s", p=P)
                    nc.sync.dma_start(out=hrc[cc], in_=zb)
                    nc.sync.dma_start(
                        out=wqT.rearrange("(c p) d -> c p d", p=P)[cc],
                        in_=zb[:, 0:HD_PER_G * D])
                    nc.sync.dma_start(
                        out=wkT.rearrange("(c p) d -> c p d", p=P)[cc],
                        in_=zb[:, 0:D])
                    nc.sync.dma_start(
                        out=wvT.rearrange("(c p) d -> c p d", p=P)[cc],
                        in_=zb[:, 0:D])
                for cc in range(HD_PER_G):
                    nc.sync.dma_start(
                        out=woT.rearrange("(m p) h -> m p h", p=P)[cc], in_=zb)
                nc.sync.dma_start(out=cosT, in_=zf1[:D, :])
                nc.sync.dma_start(out=sinT, in_=zf0[:D, :])
                nc.sync.dma_start(out=maskT, in_=zb[:, 0:P])
                nc.sync.dma_start(out=ident, in_=zb[:, 0:P])

            loop_cm = (tc.For_i(0, loop_n, 1) if loop_n is not None
                       else contextlib.nullcontext())
            with loop_cm:
              for _rep in range(repeat):
                # ---- per s-chunk: projections then attention ---------
                for c in range(WC):
                    ssl = slice(c * W, (c + 1) * W)
                    hre = hT.rearrange("(k p) s -> p k s", p=P)
                    h_sb = hpool.tile([P, HIDC, W], bf16, tag="h")
                    nc.sync.dma_start(out=h_sb, in_=hre[:, :, ssl])

                    def proj(w_sb, dsl):
                        ps = ps512.tile([P, W], f32, tag="ps512")
                        for k in range(HIDC):
                            nc.tensor.matmul(
                                ps, w_sb[:, k, dsl], h_sb[:, k, :],
                                start=(k == 0), stop=(k == HIDC - 1))
                        return ps

                    for hd in range(HD_PER_G):   # Q
                        ps = proj(wqT_sb, slice(hd * D, (hd + 1) * D))
                        rope_chunk(ps, qrT_sb[:, hd, ssl], c)
                    ps = proj(wkT_sb, slice(0, D))          # K
                    rope_chunk(ps, krT_sb[:, ssl], c)
                    ps = proj(wvT_sb, slice(0, D))          # V
                    nc.vector.tensor_copy(vT_sb[:, ssl], ps)

                    # v transposes for this chunk's new sj blocks
                    for jb in range(c * (W // P), (c + 1) * (W // P)):
                        pst = psx.tile([P, P], bf16, tag="psx")
                        nc.tensor.transpose(
                            pst, vT_sb[:, jb * P:(jb + 1) * P], ident_sb)
                        nc.vector.tensor_copy(v_nat[:, jb, :], pst)

                    # attention for si-chunk c (all K/V chunks <= c ready)
                    base = c * W
                    jbmax = (c + 1) * (W // P)
                    for hd in range(HD_PER_G):
                        avT = avtp.tile([P, W], f32, tag="avt")
                        dsum = dsump.tile([P, W], f32, tag="dsum")
                        for jb in range(jbmax):
                            si_start = max(base, jb * P)
                            off = si_start - base
                            wd = W - off
                            pss = ps512.tile([P, W], f32, tag="ps512")
                            nc.tensor.matmul(
                                pss[:, :wd],
                                krT_sb[:, jb * P:(jb + 1) * P],
                                qrT_sb[:, hd, si_start:base + W],
                                start=True, stop=True)
                            et = expp.tile([P, W], bf16, tag="expT")
                            nc.scalar.activation(
                                et[:, :wd], pss[:, :wd],
                                func=mybir.ActivationFunctionType.Exp,
                                scale=inv_sqrt_d)
                            if jb * P >= base:  # diagonal block: causal mask
                                nc.vector.tensor_mul(et[:, 0:P], et[:, 0:P], mask_sb)
                            # dsum accumulation: alternate DVE / GPSIMD
                            eng = nc.vector if jb % 2 else nc.gpsimd
                            if jb == 0:
                                eng.tensor_copy(dsum, et)
                            else:
                                eng.tensor_add(
                                    dsum[:, off:], dsum[:, off:], et[:, :wd])
                            nc.tensor.matmul(
                                avT[:, off:], v_nat[:, jb, :], et[:, :wd],
                                start=(jb == 0), stop=(jb == jbmax - 1),
                                skip_group_check=True)
                        den = rcp.tile([P, W], f32, tag="den")
                        nc.gpsimd.partition_all_reduce(den, dsum, P, ReduceOp.add)
                        nc.vector.reciprocal(den, den)
                        nc.vector.tensor_mul(
                            xT_sb[:, hd, base:base + W], avT, den)

                    # ---- output projection for this chunk's s-blocks -
                    for sb in range(c * (W // P), (c + 1) * (W // P)):
                        out_t = outp.tile([P, HID], f32, tag="out")
                        for j in range(HID // W):
                            pso = ps512.tile([P, W], f32, tag="ps512")
                            for m in range(HD_PER_G):
                                nc.tensor.matmul(
                                    pso, xT_sb[:, m, sb * P:(sb + 1) * P],
                                    woT_sb[:, m, j * W:(j + 1) * W],
                                    start=(m == 0), stop=(m == HD_PER_G - 1))
                            nc.scalar.copy(out_t[:, j * W:(j + 1) * W], pso)
                        nc.sync.dma_start(
                            out=out[sb * P:(sb + 1) * P, :], in_=out_t)

            if sink is not None:
                nc.sync.dma_start(out=sink, in_=out[0:P, 0:4])


    nc.compile()
    return nc


def _prep_inputs(hidden_states, cos, sin, wq, wk, wv, wo):
    """Host-side shard + layout prep. Returns in_maps for cores 0..7."""
    hidden_states = np.asarray(hidden_states, dtype=np.float32)
    cos = np.asarray(cos, dtype=np.float32)
    sin = np.asarray(sin, dtype=np.float32)
    wq = np.asarray(wq, dtype=np.float32)
    wk = np.asarray(wk, dtype=np.float32)
    wv = np.asarray(wv, dtype=np.float32)
    wo = np.asarray(wo, dtype=np.float32)

    cosT = np.ascontiguousarray(cos[:, 0, :].T)                # [D, S] f32
    sinT_full = sin[:, 0, :].T                                  # [D, S]
    sinT = np.concatenate([-sinT_full[:64], sinT_full[64:]], axis=0)
    sinT = np.ascontiguousarray(sinT.astype(np.float32))

    mask = (np.arange(P)[:, None] <= np.arange(P)[None, :]).astype(BF16)
    identity = np.eye(P, dtype=BF16)

    hTs = [np.ascontiguousarray(hidden_states[b].T).astype(BF16)
           for b in range(B)]

    in_maps = []
    for core in range(N_CORES):
        b, g = divmod(core, GROUPS)
        qsl = slice(g * HD_PER_G * D, (g + 1) * HD_PER_G * D)
        ksl = slice(g * D, (g + 1) * D)
        in_maps.append({
            "hT": hTs[b],
            "wqT": np.ascontiguousarray(wq[qsl, :].T).astype(BF16),
            "wkT": np.ascontiguousarray(wk[ksl, :].T).astype(BF16),
            "wvT": np.ascontiguousarray(wv[ksl, :].T).astype(BF16),
            "woT": np.ascontiguousarray(wo[:, qsl].T).astype(BF16),
            "cosT": cosT,
            "sinT": sinT,
            "maskT": mask,
            "ident": identity,
        })
    return in_maps


def kernel(hidden_states, cos, sin, wq, wk, wv, wo):
    from concourse.bass_utils import run_bass_kernel_spmd

    if "nc" not in _CACHE:
        _CACHE["nc"] = build_nc()
    nc = _CACHE["nc"]

    in_maps = _prep_inputs(hidden_states, cos, sin, wq, wk, wv, wo)
    res = run_bass_kernel_spmd(nc, in_maps, core_ids=list(range(N_CORES)))

    out = np.zeros((B, S, HID), dtype=np.float32)
    for core in range(N_CORES):
        b = core // GROUPS
        out[b] += res.results[core]["out"]
    return out


# revision 12
# speedup vs baseline: 14348.3016x; 14348.3016x over previous
"""GQA causal attention (B=2,S=2048,HID=2048,H=16,KVH=4,D=128) on 8 trn2 cores.

Sharding: core = b*4 + g  (b: batch, g: head-group of 4 Q heads + 1 KV head).
Per-core kernel computes q/k/v projections (+RoPE), causal softmax attention
for its 4 heads, and a partial output projection; host sums the 4 partials
per batch.

Layout strategy (all matmuls bf16 with fp32 PSUM accumulation):
  - hiddenT [HID, S] per batch; weights pre-transposed on host.
  - Projections (N=1024 streams) produce qT/kT/vT [dims, S]; RoPE applied in
    qT layout (rotate-half = partition-shifted ACT copy).
  - scoresT[sj, si] = kT_slice.T @ qT -> ACT exp (1/sqrt(D) folded into the
    activation scale; no max-subtraction: scores are O(+-10) so fp32 exp is
    safe) -> bf16 expT tiles.
  - AV with v stationary: avT[d, si] += v[sj,:].T @ expT[sj, si] — N<=1024
    streams, result lands directly in the xT layout needed by the output
    projection (no transposes).
  - softmax denominators: DVE-accumulate expT over sj tiles, then
    gpsimd.partition_all_reduce (replicated row sums) + reciprocal + mul.
  - out_p[s, :] = sum_m xT[m, s-block].T @ woT[m, :]  (N=1024).
"""

import math
import numpy as np
import ml_dtypes

B, S, HID = 2, 2048, 2048
H, KVH, D = 16, 4, 128
GROUPS = 4            # head groups == KV heads
HD_PER_G = 4          # query heads per group
N_CORES = 8
P = 128
HIDC = HID // P       # 16 hid chunks
W = 512               # matmul chunk (psum bank limit: 512 f32)
WC = S // W           # 4 s-chunks
SB = S // P           # 16 s-blocks of 128

BF16 = ml_dtypes.bfloat16
_CACHE = {}


def build_nc(repeat=1, loop_n=None, internal_inputs=False):
    """loop_n: if set, wrap the body in a hardware For_i loop (for timing).
    internal_inputs: declare inputs as internal DRAM (garbage data, no host
    upload) — timing-only variant."""
    import contextlib
    import concourse.bass as bass
    import concourse.tile as tile
    from concourse import bacc, mybir
    from concourse.bass_isa import ReduceOp

    f32 = mybir.dt.float32
    bf16 = mybir.dt.bfloat16

    nc = bacc.Bacc("TRN2", target_bir_lowering=False, debug=False,
                   num_devices=N_CORES)

    def din(name, shape, dt):
        if internal_inputs:
            return nc.dram_tensor(name, shape, dt).ap()
        return nc.dram_tensor(name, shape, dt, kind="ExternalInput").ap()
    hT = din("hT", [HID, S], bf16)
    wqT = din("wqT", [HID, HD_PER_G * D], bf16)
    wkT = din("wkT", [HID, D], bf16)
    wvT = din("wvT", [HID, D], bf16)
    woT = din("woT", [HD_PER_G * D, HID], bf16)
    cosT = din("cosT", [D, S], f32)
    sinT = din("sinT", [D, S], f32)
    maskT = din("maskT", [P, P], bf16)
    ident = din("ident", [P, P], bf16)
    if internal_inputs:
        out = nc.dram_tensor("out", [S, HID], f32).ap()
        sink = nc.dram_tensor("sink", [P, 4], f32, kind="ExternalOutput").ap()
    else:
        out = nc.dram_tensor("out", [S, HID], f32, kind="ExternalOutput").ap()
        sink = None

    inv_sqrt_d = 1.0 / math.sqrt(D)

    with tile.TileContext(nc) as tc:
        with (
            tc.tile_pool(name="consts", bufs=1) as consts,
            tc.tile_pool(name="persist", bufs=1) as persist,
            tc.tile_pool(name="hpool", bufs=2) as hpool,
            tc.tile_pool(name="rope", bufs=2) as rope,
            tc.tile_pool(name="expp", bufs=6) as expp,
            tc.tile_pool(name="dsump", bufs=2) as dsump,
            tc.tile_pool(name="rcp", bufs=1) as rcp,
            tc.tile_pool(name="outp", bufs=2) as outp,
            tc.tile_pool(name="ps512", bufs=5, space="PSUM") as ps512,
            tc.tile_pool(name="avt", bufs=2, space="PSUM") as avtp,
            tc.tile_pool(name="psx", bufs=1, space="PSUM") as psx,
        ):
            # ---- constant loads --------------------------------------
            wqT_sb = consts.tile([P, HIDC, HD_PER_G * D], bf16)
            nc.sync.dma_start(out=wqT_sb, in_=wqT.rearrange("(c p) d -> p c d", p=P))
            wkT_sb = consts.tile([P, HIDC, D], bf16)
            nc.sync.dma_start(out=wkT_sb, in_=wkT.rearrange("(c p) d -> p c d", p=P))
            wvT_sb = consts.tile([P, HIDC, D], bf16)
            nc.sync.dma_start(out=wvT_sb, in_=wvT.rearrange("(c p) d -> p c d", p=P))
            cosT_sb = consts.tile([P, S], f32)
            nc.sync.dma_start(out=cosT_sb, in_=cosT)
            sinT_sb = consts.tile([P, S], f32)
            nc.sync.dma_start(out=sinT_sb, in_=sinT)
            mask_sb = consts.tile([P, P], bf16)
            nc.sync.dma_start(out=mask_sb, in_=maskT)
            ident_sb = consts.tile([P, P], bf16)
            nc.sync.dma_start(out=ident_sb, in_=ident)
            woT_sb = consts.tile([P, HD_PER_G, HID], bf16)
            nc.sync.dma_start(out=woT_sb, in_=woT.rearrange("(m p) h -> p m h", p=P))

            # ---- persistent intermediates ----------------------------
            qrT_sb = persist.tile([P, HD_PER_G, S], bf16)   # rotated qT per head
            krT_sb = persist.tile([P, S], bf16)             # rotated kT
            vT_sb = persist.tile([P, S], bf16)              # vT (pre-transpose)
            v_nat = persist.tile([P, SB, D], bf16)          # v natural [sj, d]
            xT_sb = persist.tile([P, HD_PER_G, S], bf16)    # attn out (transposed)

            def rope_chunk(ps, dst_ap, c):
                """dst = ps*cos + rot_half(ps)*sin_signed on wide chunk c."""
                sl = slice(c * W, (c + 1) * W)
                t1 = rope.tile([P, W], f32, tag="t1")
                nc.vector.tensor_mul(t1, ps, cosT_sb[:, sl])
                t2 = rope.tile([P, W], f32, tag="t2")
                nc.vector.tensor_copy(t2[0:64, :], ps[64:128, :])
                nc.vector.tensor_copy(t2[64:128, :], ps[0:64, :])
                nc.vector.tensor_mul(t2, t2, sinT_sb[:, sl])
                nc.vector.tensor_add(dst_ap, t1, t2)

            if internal_inputs:
                # timing-only: fill internal inputs with finite values
                zb = consts.tile([P, S], bf16, tag="zb")
                nc.vector.memset(zb, 0.01)
                zf1 = consts.tile([P, S], f32, tag="zf1")
                nc.vector.memset(zf1, 1.0)
                zf0 = consts.tile([P, S], f32, tag="zf0")
                nc.vector.memset(zf0, 0.0)
                for cc in range(HIDC):
                    hrc = hT.rearrange("(c p) s -> c p s", p=P)
                    nc.sync.dma_start(out=hrc[cc], in_=zb)
                    nc.sync.dma_start(
                        out=wqT.rearrange("(c p) d -> c p d", p=P)[cc],
                        in_=zb[:, 0:HD_PER_G * D])
                    nc.sync.dma_start(
                        out=wkT.rearrange("(c p) d -> c p d", p=P)[cc],
                        in_=zb[:, 0:D])
                    nc.sync.dma_start(
                        out=wvT.rearrange("(c p) d -> c p d", p=P)[cc],
                        in_=zb[:, 0:D])
                for cc in range(HD_PER_G):
                    nc.sync.dma_start(
                        out=woT.rearrange("(m p) h -> m p h", p=P)[cc], in_=zb)
                nc.sync.dma_start(out=cosT, in_=zf1[:D, :])
                nc.sync.dma_start(out=sinT, in_=zf0[:D, :])
                nc.sync.dma_start(out=maskT, in_=zb[:, 0:P])
                nc.sync.dma_start(out=ident, in_=zb[:, 0:P])

            loop_cm = (tc.For_i(0, loop_n, 1) if loop_n is not None
                       else contextlib.nullcontext())
            with loop_cm:
              for _rep in range(repeat):
                # ---- per s-chunk: projections then attention ---------
                for c in range(WC):
                    ssl = slice(c * W, (c + 1) * W)
                    hre = hT.rearrange("(k p) s -> p k s", p=P)
                    h_sb = hpool.tile([P, HIDC, W], bf16, tag="h")
                    nc.sync.dma_start(out=h_sb, in_=hre[:, :, ssl])

                    def proj(w_sb, dsl):
                        ps = ps512.tile([P, W], f32, tag="ps512")
                        for k in range(HIDC):
                            nc.tensor.matmul(
                                ps, w_sb[:, k, dsl], h_sb[:, k, :],
                                start=(k == 0), stop=(k == HIDC - 1))
                        return ps

                    for hd in range(HD_PER_G):   # Q
                        ps = proj(wqT_sb, slice(hd * D, (hd + 1) * D))
                        rope_chunk(ps, qrT_sb[:, hd, ssl], c)
                    ps = proj(wkT_sb, slice(0, D))          # K
                    rope_chunk(ps, krT_sb[:, ssl], c)
                    ps = proj(wvT_sb, slice(0, D))          # V
                    nc.vector.tensor_copy(vT_sb[:, ssl], ps)

                    # v transposes for this chunk's new sj blocks
                    for jb in range(c * (W // P), (c + 1) * (W // P)):
                        pst = psx.tile([P, P], bf16, tag="psx")
                        nc.tensor.transpose(
                            pst, vT_sb[:, jb * P:(jb + 1) * P], ident_sb)
                        nc.vector.tensor_copy(v_nat[:, jb, :], pst)

                    # attention for si-chunk c (all K/V chunks <= c ready)
                    base = c * W
                    jbmax = (c + 1) * (W // P)
                    for hd in range(HD_PER_G):
                        avT = avtp.tile([P, W], f32, tag="avt")
                        dsum = dsump.tile([P, W], f32, tag="dsum")
                        for jb in range(jbmax):
                            si_start = max(base, jb * P)
                            off = si_start - base
                            wd = W - off
                            pss = ps512.tile([P, W], f32, tag="ps512")
                            nc.tensor.matmul(
                                pss[:, :wd],
                                krT_sb[:, jb * P:(jb + 1) * P],
                                qrT_sb[:, hd, si_start:base + W],
                                start=True, stop=True)
                            et = expp.tile([P, W], bf16, tag="expT")
                            nc.scalar.activation(
                                et[:, :wd], pss[:, :wd],
                                func=mybir.ActivationFunctionType.Exp,
                                scale=inv_sqrt_d)
                            if jb * P >= base:  # diagonal block: causal mask
                                nc.vector.tensor_mul(et[:, 0:P], et[:, 0:P], mask_sb)
                            if jb == 0:
                                nc.vector.tensor_copy(dsum, et)
                            else:
                                nc.vector.tensor_add(
                                    dsum[:, off:], dsum[:, off:], et[:, :wd])
                            nc.tensor.matmul(
                                avT[:, off:], v_nat[:, jb, :], et[:, :wd],
                                start=(jb == 0), stop=(jb == jbmax - 1),
                                skip_group_check=True)
                        den = rcp.tile([P, W], f32, tag="den")
                        nc.gpsimd.partition_all_reduce(den, dsum, P, ReduceOp.add)
                        nc.vector.reciprocal(den, den)
                        nc.vector.tensor_mul(
                            xT_sb[:, hd, base:base + W], avT, den)

                    # ---- output projection for this chunk's s-blocks -
                    for sb in range(c * (W // P), (c + 1) * (W // P)):
                        out_t = outp.tile([P, HID], f32, tag="out")
                        for j in range(HID // W):
                            pso = ps512.tile([P, W], f32, tag="ps512")
                            for m in range(HD_PER_G):
                                nc.tensor.matmul(
                                    pso, xT_sb[:, m, sb * P:(sb + 1) * P],
                                    woT_sb[:, m, j * W:(j + 1) * W],
                                    start=(m == 0), stop=(m == HD_PER_G - 1))
                            nc.scalar.copy(out_t[:, j * W:(j + 1) * W], pso)
                        nc.sync.dma_start(
                            out=out[sb * P:(sb + 1) * P, :], in_=out_t)

            if sink is not None:
                nc.sync.dma_start(out=sink, in_=out[0:P, 0:4])


    nc.compile()
    return nc


def _prep_inputs(hidden_states, cos, sin, wq, wk, wv, wo):
    """Host-side shard + layout prep. Returns in_maps for cores 0..7."""
    hidden_states = np.asarray(hidden_states, dtype=np.float32)
    cos = np.asarray(cos, dtype=np.float32)
    sin = np.asarray(sin, dtype=np.float32)
    wq = np.asarray(wq, dtype=np.float32)
    wk = np.asarray(wk, dtype=np.float32)
    wv = np.asarray(wv, dtype=np.float32)
    wo = np.asarray(wo, dtype=np.float32)

    cosT = np.ascontiguousarray(cos[:, 0, :].T)                # [D, S] f32
    sinT_full = sin[:, 0, :].T                                  # [D, S]
    sinT = np.concatenate([-sinT_full[:64], sinT_full[64:]], axis=0)
    sinT = np.ascontiguousarray(sinT.astype(np.float32))

    mask = (np.arange(P)[:, None] <= np.arange(P)[None, :]).astype(BF16)
    identity = np.eye(P, dtype=BF16)

    hTs = [np.ascontiguousarray(hidden_states[b].T).astype(BF16)
           for b in range(B)]

    in_maps = []
    for core in range(N_CORES):
        b, g = divmod(core, GROUPS)
        qsl = slice(g * HD_PER_G * D, (g + 1) * HD_PER_G * D)
        ksl = slice(g * D, (g + 1) * D)
        in_maps.append({
            "hT": hTs[b],
            "wqT": np.ascontiguousarray(wq[qsl, :].T).astype(BF16),
            "wkT": np.ascontiguousarray(wk[ksl, :].T).astype(BF16),
            "wvT": np.ascontiguousarray(wv[ksl, :].T).astype(BF16),
            "woT": np.ascontiguousarray(wo[:, qsl].T).astype(BF16),
            "cosT": cosT,
            "sinT": sinT,
            "maskT": mask,
            "ident": identity,
        })
    return in_maps


def kernel(hidden_states, cos, sin, wq, wk, wv, wo):
    from concourse.bass_utils import run_bass_kernel_spmd

    if "nc" not in _CACHE:
        _CACHE["nc"] = build_nc()
    nc = _CACHE["nc"]

    in_maps = _prep_inputs(hidden_states, cos, sin, wq, wk, wv, wo)
    res = run_bass_kernel_spmd(nc, in_maps, core_ids=list(range(N_CORES)))

    out = np.zeros((B, S, HID), dtype=np.float32)
    for core in range(N_CORES):
        b = core // GROUPS
        out[b] += res.results[core]["out"]
    return out
